# revision 7
# baseline (speedup 1.0000x reference)
"""Trainium2 Bass kernel for GQA attention (B=2, S=2048, D=2048, 16 q-heads /
4 kv-heads, HD=128) with per-head QK RMSNorm + RoPE + causal softmax + output
projection.

Sharding: 8 cores = (batch b in {0,1}) x (kv-group g in {0..3}). Each core
computes its batch's 4 q-heads + 1 kv-head and a partial output through the
row-sharded Wo; the host sums the 4 partials per batch.
"""
import numpy as np

import concourse.bass as bass  # noqa: F401  (bass types used via bacc/tile)
import concourse.mybir as mybir
import concourse.tile as tile
from concourse import bacc
from concourse.bass_utils import run_bass_kernel_spmd

F32 = mybir.dt.float32
F32R = mybir.dt.float32r
AF = mybir.ActivationFunctionType
OP = mybir.AluOpType

B, S, D = 2, 2048, 2048
NH, NKV, HD = 16, 4, 128
REP = NH // NKV
EPS = 1e-6
NEG = -1.0e30


def build(s=S):
    """Build + compile the per-core SPMD program (identical on all 8 cores)."""
    sc = s // 128          # s-chunks
    kc = D // 128          # contraction chunks
    nsb = sc // 4          # q superblocks (512 wide)
    nc = bacc.Bacc("TRN2", target_bir_lowering=False, debug=False, num_devices=8)

    xT_d = nc.dram_tensor("xT", [D, s], F32R, kind="ExternalInput")
    wqkv_d = nc.dram_tensor("wqkv", [D, 768], F32R, kind="ExternalInput")
    wo_d = nc.dram_tensor("wo", [512, D], F32R, kind="ExternalInput")
    cwq_d = nc.dram_tensor("cwq", [s, HD], F32, kind="ExternalInput")
    swq_d = nc.dram_tensor("swq", [s, HD], F32, kind="ExternalInput")
    cwk_d = nc.dram_tensor("cwk", [s, HD], F32, kind="ExternalInput")
    swk_d = nc.dram_tensor("swk", [s, HD], F32, kind="ExternalInput")
    mask_d = nc.dram_tensor("maskb", [128, 128], F32, kind="ExternalInput")
    zero_d = nc.dram_tensor("zeros", [128, 384], F32R, kind="ExternalInput")
    iden_d = nc.dram_tensor("ident", [128, 128], F32R, kind="ExternalInput")
    out_d = nc.dram_tensor("outp", [s, D], F32, kind="ExternalOutput")

    with tile.TileContext(nc) as tc:
        with (
            tc.tile_pool(name="pers", bufs=1) as pers,
            tc.tile_pool(name="psA", bufs=3, space="PSUM") as psA,
            tc.tile_pool(name="psB", bufs=2, space="PSUM") as psB,
            tc.tile_pool(name="psT", bufs=3, space="PSUM") as psT,
        ):
            qT = pers.tile([128, REP, s], F32R, tag="qT")
            kT = pers.tile([128, s], F32R, tag="kT")
            vv = pers.tile([128, sc, HD], F32R, tag="vv")
            aoT = pers.tile([128, REP, s], F32R, tag="aoT")
            mask_t = pers.tile([128, 128], F32, tag="maskb")
            iden_t = pers.tile([128, 128], F32R, tag="ident")
            nc.sync.dma_start(out=mask_t[:], in_=mask_d[:, :])
            nc.sync.dma_start(out=iden_t[:], in_=iden_d[:, :])
            eps_t = pers.tile([128, 1], F32, tag="eps")
            nc.vector.memset(eps_t[:], EPS)
            zero_t = pers.tile([128, 384], F32R, tag="zeros")
            nc.sync.dma_start(out=zero_t[:], in_=zero_d[:, :])

            # ---------------- Phase 1: QKV + RMSNorm + RoPE -----------------
            with (
                tc.tile_pool(name="wq", bufs=1) as wq,
                tc.tile_pool(name="xp", bufs=2) as xp,
                tc.tile_pool(name="cp", bufs=2) as cp,
                tc.tile_pool(name="st", bufs=3) as st,
            ):
                wqkv_t = wq.tile([128, kc, 768], F32R, tag="wqkv")
                nc.sync.dma_start(
                    out=wqkv_t[:],
                    in_=wqkv_d.rearrange("(dk ki) e -> ki dk e", ki=128),
                )
                cwq_r = cwq_d.rearrange("(m si) h -> si m h", si=128)
                swq_r = swq_d.rearrange("(m si) h -> si m h", si=128)
                cwk_r = cwk_d.rearrange("(m si) h -> si m h", si=128)
                swk_r = swk_d.rearrange("(m si) h -> si m h", si=128)
                xT_r = xT_d.rearrange("(dk ki) t -> ki dk t", ki=128)

                for m in range(sc):
                    xt = xp.tile([128, kc, 128], F32R, tag="xt")
                    nc.sync.dma_start(out=xt[:], in_=xT_r[:, :, m * 128:(m + 1) * 128])
                    cq = cp.tile([128, 128], F32, tag="cq")
                    sq_ = cp.tile([128, 128], F32, tag="sq_")
                    ck = cp.tile([128, 128], F32, tag="ck")
                    sk_ = cp.tile([128, 128], F32, tag="sk_")
                    nc.sync.dma_start(out=cq[:], in_=cwq_r[:, m])
                    nc.sync.dma_start(out=sq_[:], in_=swq_r[:, m])
                    nc.sync.dma_start(out=ck[:], in_=cwk_r[:, m])
                    nc.sync.dma_start(out=sk_[:], in_=swk_r[:, m])

                    pq = psA.tile([128, 512], F32, tag="psA")
                    pkv = psB.tile([128, 512], F32, tag="psB")
                    for k in range(kc):
                        nc.tensor.matmul(
                            pq[:], xt[:, k], wqkv_t[:, k, 0:512],
                            start=(k == 0), stop=(k == kc - 1),
                        )
                    for k in range(kc):
                        nc.tensor.matmul(
                            pkv[:, 0:256], xt[:, k], wqkv_t[:, k, 512:768],
                            start=(k == 0), stop=(k == kc - 1),
                        )

                    ss = st.tile([128, 16], F32, tag="ss")
                    rs = st.tile([128, 8], F32, tag="rs")
                    # per-head q norm + rope, k norm + rope
                    for h in range(REP + 1):
                        is_k = h == REP
                        src = pkv[:, 0:128] if is_k else pq[:, h * 128:(h + 1) * 128]
                        srcA = pkv[:, 64:128] if is_k else pq[:, h * 128 + 64:h * 128 + 128]
                        srcB = pkv[:, 0:64] if is_k else pq[:, h * 128:h * 128 + 64]
                        cw = ck if is_k else cq
                        sw = sk_ if is_k else sq_
                        sqs = st.tile([128, 128], F32, tag="sqs")
                        nc.scalar.activation(
                            sqs[:], src, AF.Square, accum_out=ss[:, h:h + 1],
                        )
                        nc.scalar.activation(
                            ss[:, h + 8:h + 9], ss[:, h:h + 1], AF.Sqrt,
                            bias=eps_t[:], scale=1.0 / HD,
                        )
                        nc.vector.reciprocal(rs[:, h:h + 1], ss[:, h + 8:h + 9])
                        ra = st.tile([128, 128], F32, tag="ra")
                        rb = st.tile([128, 128], F32, tag="rb")
                        nc.vector.scalar_tensor_tensor(
                            out=ra[:], in0=src, scalar=rs[:, h:h + 1], in1=cw[:],
                            op0=OP.mult, op1=OP.mult,
                        )
                        nc.vector.scalar_tensor_tensor(
                            out=rb[:, 0:64], in0=srcA, scalar=rs[:, h:h + 1],
                            in1=sw[:, 0:64], op0=OP.mult, op1=OP.mult,
                        )
                        nc.vector.scalar_tensor_tensor(
                            out=rb[:, 64:128], in0=srcB, scalar=rs[:, h:h + 1],
                            in1=sw[:, 64:128], op0=OP.mult, op1=OP.mult,
                        )
                        qn = st.tile([128, 128], F32R, tag="qn")
                        nc.vector.tensor_add(qn[:], ra[:], rb[:])
                        pt = psT.tile([128, 512], F32R, tag="psT")
                        nc.tensor.transpose(pt[:, 0:128], qn[:], iden_t[:])
                        dst = kT[:, m * 128:(m + 1) * 128] if is_k else \
                            qT[:, h, m * 128:(m + 1) * 128]
                        nc.any.tensor_copy(out=dst, in_=pt[:, 0:128])
                    nc.any.tensor_copy(out=vv[:, m, :], in_=pkv[:, 128:256])

            # ---------------- Phase 2: causal attention ---------------------
            with (
                tc.tile_pool(name="pp", bufs=2) as pp,
                tc.tile_pool(name="tb", bufs=3) as tb,
                tc.tile_pool(name="ap", bufs=4) as ap,
            ):
                for h in range(REP):
                    for Q in range(nsb):
                        ptiles = {}
                        for i in range(4 * Q, 4 * Q + 4):
                            klen = (i + 1) * 128
                            nblk = (klen + 511) // 512
                            probs = pp.tile([128, s], F32R, tag=f"probs{i % 4}")
                            ptiles[i % 4] = probs
                            acc = ap.tile([128, 4], F32, tag="acc")
                            for kb in range(nblk):
                                cols = min(512, klen - kb * 512)
                                ps_s = psA.tile([128, 512], F32, tag="psA")
                                nc.tensor.matmul(
                                    ps_s[:, :cols],
                                    qT[:, h, i * 128:(i + 1) * 128],
                                    kT[:, kb * 512:kb * 512 + cols],
                                    start=True, stop=True,
                                )
                                if kb == i // 4:
                                    dc = i * 128 - kb * 512
                                    nc.vector.tensor_add(
                                        ps_s[:, dc:dc + 128],
                                        ps_s[:, dc:dc + 128], mask_t[:],
                                    )
                                nc.scalar.activation(
                                    probs[:, kb * 512:kb * 512 + cols],
                                    ps_s[:, :cols], AF.Exp,
                                    accum_out=acc[:, kb:kb + 1],
                                )
                            rec = ap.tile([128, 1], F32, tag="rec")
                            if nblk > 1:
                                sm = ap.tile([128, 1], F32, tag="sm")
                                nc.vector.tensor_reduce(
                                    out=sm[:], in_=acc[:, :nblk],
                                    axis=mybir.AxisListType.X, op=OP.add,
                                )
                                nc.vector.reciprocal(rec[:], sm[:])
                            else:
                                nc.vector.reciprocal(rec[:], acc[:, 0:1])
                            nc.vector.tensor_scalar_mul(
                                probs[:, :klen], probs[:, :klen], rec[:],
                            )
                        pvp = psB.tile([128, 512], F32, tag="psB")
                        for j in range(4 * Q + 4):
                            c0 = (j - 4 * Q) * 128 if j > 4 * Q else 0
                            ptps = psT.tile([128, 512], F32R, tag="psT")
                            for i in range(max(4 * Q, j), 4 * Q + 4):
                                cc = (i - 4 * Q) * 128
                                nc.tensor.transpose(
                                    ptps[:, cc:cc + 128],
                                    ptiles[i % 4][:, j * 128:(j + 1) * 128],
                                    iden_t[:],
                                )
                            ptT = tb.tile([128, 512], F32R, tag="ptT")
                            if c0 > 0:
                                nc.any.tensor_copy(out=ptT[:, :c0], in_=zero_t[:, :c0])
                            nc.any.tensor_copy(out=ptT[:, c0:], in_=ptps[:, c0:])
                            nc.tensor.matmul(
                                pvp[:], vv[:, j, :], ptT[:],
                                start=(j == 0), stop=(j == 4 * Q + 3),
                            )
                        nc.any.tensor_copy(
                            out=aoT[:, h, Q * 512:(Q + 1) * 512], in_=pvp[:],
                        )

            # ---------------- Phase 3: output projection --------------------
            with (
                tc.tile_pool(name="wop", bufs=1) as wop,
                tc.tile_pool(name="ob", bufs=2) as ob,
            ):
                wo_t = wop.tile([128, REP, D], F32R, tag="wo")
                nc.sync.dma_start(
                    out=wo_t[:], in_=wo_d.rearrange("(e ki) d -> ki e d", ki=128),
                )
                for m in range(sc):
                    ot = ob.tile([128, D], F32, tag="ot")
                    for n in range(D // 512):
                        po = psA.tile([128, 512], F32, tag="psA")
                        for e in range(REP):
                            nc.tensor.matmul(
                                po[:], aoT[:, e, m * 128:(m + 1) * 128],
                                wo_t[:, e, n * 512:(n + 1) * 512],
                                start=(e == 0), stop=(e == REP - 1),
                            )
                        nc.any.tensor_copy(out=ot[:, n * 512:(n + 1) * 512], in_=po[:])
                    nc.sync.dma_start(
                        out=out_d[m * 128:(m + 1) * 128, :], in_=ot[:],
                    )

    nc.compile()
    return nc


def make_in_maps(x, cos, sin, Wq, Wk, Wv, Wo, q_norm_w, k_norm_w):
    s = x.shape[1]
    qsc = (q_norm_w / np.sqrt(HD)).astype(np.float32)
    ksc = k_norm_w.astype(np.float32)

    def rope_consts(w):
        cw = (cos * w[None, :]).astype(np.float32)
        sw = np.empty_like(cw)
        sw[:, :64] = -sin[:, :64] * w[None, 64:]
        sw[:, 64:] = sin[:, 64:] * w[None, :64]
        return cw, sw

    cwq, swq = rope_consts(qsc)
    cwk, swk = rope_consts(ksc)
    r = np.arange(128)
    maskb = np.where(r[None, :] > r[:, None], NEG, 0.0).astype(np.float32)
    ident = np.eye(128, dtype=np.float32)

    in_maps = []
    for c in range(8):
        b, g = c // 4, c % 4
        xT = np.ascontiguousarray(x[b].T)
        wqkv = np.ascontiguousarray(
            np.concatenate(
                [
                    Wq[:, g * 512:(g + 1) * 512],
                    Wk[:, g * 128:(g + 1) * 128],
                    Wv[:, g * 128:(g + 1) * 128],
                ],
                axis=1,
            )
        )
        wo = np.ascontiguousarray(Wo[g * 512:(g + 1) * 512, :])
        in_maps.append(
            dict(
                xT=xT, wqkv=wqkv, wo=wo, cwq=cwq, swq=swq, cwk=cwk, swk=swk,
                maskb=maskb, ident=ident, zeros=np.zeros((128, 384), np.float32),
            )
        )
    return in_maps


_cached = None


def kernel(x, cos, sin, Wq, Wk, Wv, Wo, q_norm_w, k_norm_w):
    global _cached
    x = np.asarray(x, np.float32)
    cos = np.asarray(cos, np.float32)
    sin = np.asarray(sin, np.float32)
    in_maps = make_in_maps(
        x, cos, sin,
        np.asarray(Wq, np.float32), np.asarray(Wk, np.float32),
        np.asarray(Wv, np.float32), np.asarray(Wo, np.float32),
        np.asarray(q_norm_w, np.float32), np.asarray(k_norm_w, np.float32),
    )
    if _cached is None:
        _cached = build()
    res = run_bass_kernel_spmd(_cached, in_maps, core_ids=list(range(8)))
    out = np.zeros((B, S, D), np.float64)
    for c in range(8):
        out[c // 4] += res.results[c]["outp"].astype(np.float64)
    return out.astype(np.float32)


# revision 8
# speedup vs baseline: 1.0533x; 1.0533x over previous
"""Trainium2 Bass kernel for GQA attention (B=2, S=2048, D=2048, 16 q-heads /
4 kv-heads, HD=128) with per-head QK RMSNorm + RoPE + causal softmax + output
projection.

Sharding: 8 cores = (batch b in {0,1}) x (kv-group g in {0..3}). Each core
computes its batch's 4 q-heads + 1 kv-head and a partial output through the
row-sharded Wo; the host sums the 4 partials per batch.
"""
import numpy as np

import concourse.bass as bass  # noqa: F401
import concourse.mybir as mybir
import concourse.tile as tile
from concourse import bacc
from concourse.bass_utils import run_bass_kernel_spmd

F32 = mybir.dt.float32
F32R = mybir.dt.float32r
F16 = mybir.dt.float16
AF = mybir.ActivationFunctionType
OP = mybir.AluOpType

B, S, D = 2, 2048, 2048
NH, NKV, HD = 16, 4, 128
REP = NH // NKV
EPS = 1e-6
NEG = -1.0e30
EXPB = -5.0  # exp bias: cancels in softmax, keeps exp() in fp16 range


def build(s=S):
    """Build + compile the per-core SPMD program (identical on all 8 cores)."""
    sc = s // 128          # s-chunks
    kc = D // 128          # contraction chunks
    nsb = sc // 4          # q superblocks (512 wide)
    nc = bacc.Bacc("TRN2", target_bir_lowering=False, debug=False, num_devices=8)

    xT_d = nc.dram_tensor("xT", [D, s], F32R, kind="ExternalInput")
    wqkv_d = nc.dram_tensor("wqkv", [D, 768], F32R, kind="ExternalInput")
    wo_d = nc.dram_tensor("wo", [512, D], F32R, kind="ExternalInput")
    cwq_d = nc.dram_tensor("cwq", [s, HD], F32, kind="ExternalInput")
    swq_d = nc.dram_tensor("swq", [s, HD], F32, kind="ExternalInput")
    cwk_d = nc.dram_tensor("cwk", [s, HD], F32, kind="ExternalInput")
    swk_d = nc.dram_tensor("swk", [s, HD], F32, kind="ExternalInput")
    mask_d = nc.dram_tensor("maskb", [128, 128], F32, kind="ExternalInput")
    iden_d = nc.dram_tensor("ident", [128, 128], F32R, kind="ExternalInput")
    iden16_d = nc.dram_tensor("ident16", [128, 128], F16, kind="ExternalInput")
    out_d = nc.dram_tensor("outp", [s, D], F32, kind="ExternalOutput")

    with tile.TileContext(nc) as tc:
        with (
            tc.tile_pool(name="pers", bufs=1) as pers,
            tc.tile_pool(name="psA", bufs=3, space="PSUM") as psA,
            tc.tile_pool(name="psB", bufs=2, space="PSUM") as psB,
            tc.tile_pool(name="psT", bufs=3, space="PSUM") as psT,
        ):
            qT = pers.tile([128, REP, s], F32R, tag="qT")
            kT = pers.tile([128, s], F32R, tag="kT")
            vv = pers.tile([128, sc, HD], F16, tag="vv")
            aoT = pers.tile([128, REP, s], F32R, tag="aoT")
            mask_t = pers.tile([128, 128], F32, tag="maskb")
            iden_t = pers.tile([128, 128], F32R, tag="ident")
            iden16_t = pers.tile([128, 128], F16, tag="ident16")
            nc.sync.dma_start(out=mask_t[:], in_=mask_d[:, :])
            nc.sync.dma_start(out=iden_t[:], in_=iden_d[:, :])
            nc.sync.dma_start(out=iden16_t[:], in_=iden16_d[:, :])
            eps_t = pers.tile([128, 1], F32, tag="eps")
            nc.vector.memset(eps_t[:], EPS)
            expb_t = pers.tile([128, 1], F32, tag="expb")
            nc.vector.memset(expb_t[:], EXPB)
            zero_t = pers.tile([128, 384], F16, tag="zeros")
            nc.vector.memset(zero_t[:], 0.0)

            # ---------------- Phase 1: QKV + RMSNorm + RoPE -----------------
            with (
                tc.tile_pool(name="wq", bufs=1) as wq,
                tc.tile_pool(name="xp", bufs=3) as xp,
                tc.tile_pool(name="cp", bufs=2) as cp,
                tc.tile_pool(name="st", bufs=3) as st,
            ):
                wqkv_t = wq.tile([128, kc, 768], F32R, tag="wqkv")
                wqkv_r = wqkv_d.rearrange("(dk ki) e -> ki dk e", ki=128)
                for k in range(kc):
                    nc.sync.dma_start(out=wqkv_t[:, k], in_=wqkv_r[:, k])
                cwq_r = cwq_d.rearrange("(m si) h -> si m h", si=128)
                swq_r = swq_d.rearrange("(m si) h -> si m h", si=128)
                cwk_r = cwk_d.rearrange("(m si) h -> si m h", si=128)
                swk_r = swk_d.rearrange("(m si) h -> si m h", si=128)
                xT_r = xT_d.rearrange("(dk ki) t -> ki dk t", ki=128)

                for m in range(sc):
                    xt = xp.tile([128, kc, 128], F32R, tag="xt")
                    for k in range(kc):
                        nc.sync.dma_start(
                            out=xt[:, k], in_=xT_r[:, k, m * 128:(m + 1) * 128],
                        )
                    cq = cp.tile([128, 128], F32, tag="cq")
                    sq_ = cp.tile([128, 128], F32, tag="sq_")
                    ck = cp.tile([128, 128], F32, tag="ck")
                    sk_ = cp.tile([128, 128], F32, tag="sk_")
                    nc.sync.dma_start(out=cq[:], in_=cwq_r[:, m])
                    nc.sync.dma_start(out=sq_[:], in_=swq_r[:, m])
                    nc.sync.dma_start(out=ck[:], in_=cwk_r[:, m])
                    nc.sync.dma_start(out=sk_[:], in_=swk_r[:, m])

                    pq = psA.tile([128, 512], F32, tag="psA")
                    pkv = psB.tile([128, 512], F32, tag="psB")
                    for k in range(kc):
                        nc.tensor.matmul(
                            pq[:], xt[:, k], wqkv_t[:, k, 0:512],
                            start=(k == 0), stop=(k == kc - 1),
                        )
                    for k in range(kc):
                        nc.tensor.matmul(
                            pkv[:, 0:256], xt[:, k], wqkv_t[:, k, 512:768],
                            start=(k == 0), stop=(k == kc - 1),
                        )

                    ss = st.tile([128, 16], F32, tag="ss")
                    rs = st.tile([128, 8], F32, tag="rs")
                    for h in range(REP + 1):
                        is_k = h == REP
                        src = pkv[:, 0:128] if is_k else pq[:, h * 128:(h + 1) * 128]
                        srcA = pkv[:, 64:128] if is_k else pq[:, h * 128 + 64:h * 128 + 128]
                        srcB = pkv[:, 0:64] if is_k else pq[:, h * 128:h * 128 + 64]
                        cw = ck if is_k else cq
                        sw = sk_ if is_k else sq_
                        sqs = st.tile([128, 128], F32, tag="sqs")
                        nc.scalar.activation(
                            sqs[:], src, AF.Square, accum_out=ss[:, h:h + 1],
                        )
                        nc.scalar.activation(
                            ss[:, h + 8:h + 9], ss[:, h:h + 1], AF.Sqrt,
                            bias=eps_t[:], scale=1.0 / HD,
                        )
                        nc.vector.reciprocal(rs[:, h:h + 1], ss[:, h + 8:h + 9])
                        ra = st.tile([128, 128], F32, tag="ra")
                        rb = st.tile([128, 128], F32, tag="rb")
                        nc.vector.scalar_tensor_tensor(
                            out=ra[:], in0=src, scalar=rs[:, h:h + 1], in1=cw[:],
                            op0=OP.mult, op1=OP.mult,
                        )
                        nc.vector.scalar_tensor_tensor(
                            out=rb[:, 0:64], in0=srcA, scalar=rs[:, h:h + 1],
                            in1=sw[:, 0:64], op0=OP.mult, op1=OP.mult,
                        )
                        nc.vector.scalar_tensor_tensor(
                            out=rb[:, 64:128], in0=srcB, scalar=rs[:, h:h + 1],
                            in1=sw[:, 64:128], op0=OP.mult, op1=OP.mult,
                        )
                        qn = st.tile([128, 128], F32R, tag="qn")
                        nc.vector.tensor_add(qn[:], ra[:], rb[:])
                        pt = psT.tile([128, 512], F32R, tag="psT")
                        nc.tensor.transpose(pt[:, 0:128], qn[:], iden_t[:])
                        dst = kT[:, m * 128:(m + 1) * 128] if is_k else \
                            qT[:, h, m * 128:(m + 1) * 128]
                        nc.vector.tensor_copy(out=dst, in_=pt[:, 0:128])
                    nc.vector.tensor_copy(out=vv[:, m, :], in_=pkv[:, 128:256])

            # ------- Phase 2+3: causal attention + fused out-projection -----
            with (
                tc.tile_pool(name="wop", bufs=1) as wop,
                tc.tile_pool(name="pp", bufs=2) as pp,
                tc.tile_pool(name="tb", bufs=3) as tb,
                tc.tile_pool(name="ap", bufs=4) as ap,
                tc.tile_pool(name="ob", bufs=2) as ob,
            ):
                wo_t = wop.tile([128, REP, D], F32R, tag="wo")
                nc.sync.dma_start(
                    out=wo_t[:], in_=wo_d.rearrange("(e ki) d -> ki e d", ki=128),
                )
                for Q in range(nsb):
                    for h in range(REP):
                        ptiles = {}
                        for i in range(4 * Q, 4 * Q + 4):
                            klen = (i + 1) * 128
                            nblk = (klen + 511) // 512
                            probs = pp.tile([128, s], F16, tag=f"probs{i % 4}")
                            ptiles[i % 4] = probs
                            acc = ap.tile([128, 4], F32, tag="acc")
                            for kb in range(nblk):
                                cols = min(512, klen - kb * 512)
                                ps_s = psA.tile([128, 512], F32, tag="psA")
                                nc.tensor.matmul(
                                    ps_s[:, :cols],
                                    qT[:, h, i * 128:(i + 1) * 128],
                                    kT[:, kb * 512:kb * 512 + cols],
                                    start=True, stop=True,
                                )
                                if kb == i // 4:
                                    dc = i * 128 - kb * 512
                                    nc.vector.tensor_add(
                                        ps_s[:, dc:dc + 128],
                                        ps_s[:, dc:dc + 128], mask_t[:],
                                    )
                                nc.scalar.activation(
                                    probs[:, kb * 512:kb * 512 + cols],
                                    ps_s[:, :cols], AF.Exp, bias=expb_t[:],
                                    accum_out=acc[:, kb:kb + 1],
                                )
                            rec = ap.tile([128, 1], F32, tag="rec")
                            if nblk > 1:
                                sm = ap.tile([128, 1], F32, tag="sm")
                                nc.vector.tensor_reduce(
                                    out=sm[:], in_=acc[:, :nblk],
                                    axis=mybir.AxisListType.X, op=OP.add,
                                )
                                nc.vector.reciprocal(rec[:], sm[:])
                            else:
                                nc.vector.reciprocal(rec[:], acc[:, 0:1])
                            nc.vector.tensor_scalar_mul(
                                probs[:, :klen], probs[:, :klen], rec[:],
                            )
                        pvp = psB.tile([128, 512], F32, tag="psB")
                        for j in range(4 * Q + 4):
                            c0 = (j - 4 * Q) * 128 if j > 4 * Q else 0
                            ptps = psT.tile([128, 512], F16, tag="psT")
                            for i in range(max(4 * Q, j), 4 * Q + 4):
                                cc = (i - 4 * Q) * 128
                                nc.tensor.transpose(
                                    ptps[:, cc:cc + 128],
                                    ptiles[i % 4][:, j * 128:(j + 1) * 128],
                                    iden16_t[:],
                                )
                            ptT = tb.tile([128, 512], F16, tag="ptT")
                            if c0 > 0:
                                nc.vector.tensor_copy(
                                    out=ptT[:, :c0], in_=zero_t[:, :c0],
                                )
                            nc.vector.tensor_copy(out=ptT[:, c0:], in_=ptps[:, c0:])
                            nc.tensor.matmul(
                                pvp[:], vv[:, j, :], ptT[:],
                                start=(j == 0), stop=(j == 4 * Q + 3),
                            )
                        nc.vector.tensor_copy(
                            out=aoT[:, h, Q * 512:(Q + 1) * 512], in_=pvp[:],
                        )
                    # fused out-projection for this superblock's s-chunks
                    for m in range(4 * Q, 4 * Q + 4):
                        ot = ob.tile([128, D], F32, tag="ot")
                        for n in range(D // 512):
                            po = psB.tile([128, 512], F32, tag="psB")
                            for e in range(REP):
                                nc.tensor.matmul(
                                    po[:], aoT[:, e, m * 128:(m + 1) * 128],
                                    wo_t[:, e, n * 512:(n + 1) * 512],
                                    start=(e == 0), stop=(e == REP - 1),
                                )
                            nc.scalar.copy(out=ot[:, n * 512:(n + 1) * 512], in_=po[:])
                        nc.sync.dma_start(
                            out=out_d[m * 128:(m + 1) * 128, :], in_=ot[:],
                        )

    nc.compile()
    return nc


def make_in_maps(x, cos, sin, Wq, Wk, Wv, Wo, q_norm_w, k_norm_w):
    qsc = (q_norm_w / np.sqrt(HD)).astype(np.float32)
    ksc = k_norm_w.astype(np.float32)

    def rope_consts(w):
        cw = (cos * w[None, :]).astype(np.float32)
        sw = np.empty_like(cw)
        sw[:, :64] = -sin[:, :64] * w[None, 64:]
        sw[:, 64:] = sin[:, 64:] * w[None, :64]
        return cw, sw

    cwq, swq = rope_consts(qsc)
    cwk, swk = rope_consts(ksc)
    r = np.arange(128)
    maskb = np.where(r[None, :] > r[:, None], NEG, 0.0).astype(np.float32)
    ident = np.eye(128, dtype=np.float32)
    ident16 = np.eye(128, dtype=np.float16)

    in_maps = []
    for c in range(8):
        b, g = c // 4, c % 4
        xT = np.ascontiguousarray(x[b].T)
        wqkv = np.ascontiguousarray(
            np.concatenate(
                [
                    Wq[:, g * 512:(g + 1) * 512],
                    Wk[:, g * 128:(g + 1) * 128],
                    Wv[:, g * 128:(g + 1) * 128],
                ],
                axis=1,
            )
        )
        wo = np.ascontiguousarray(Wo[g * 512:(g + 1) * 512, :])
        in_maps.append(
            dict(
                xT=xT, wqkv=wqkv, wo=wo, cwq=cwq, swq=swq, cwk=cwk, swk=swk,
                maskb=maskb, ident=ident, ident16=ident16,
            )
        )
    return in_maps


_cached = None


def kernel(x, cos, sin, Wq, Wk, Wv, Wo, q_norm_w, k_norm_w):
    global _cached
    x = np.asarray(x, np.float32)
    cos = np.asarray(cos, np.float32)
    sin = np.asarray(sin, np.float32)
    in_maps = make_in_maps(
        x, cos, sin,
        np.asarray(Wq, np.float32), np.asarray(Wk, np.float32),
        np.asarray(Wv, np.float32), np.asarray(Wo, np.float32),
        np.asarray(q_norm_w, np.float32), np.asarray(k_norm_w, np.float32),
    )
    if _cached is None:
        _cached = build()
    res = run_bass_kernel_spmd(_cached, in_maps, core_ids=list(range(8)))
    out = np.zeros((B, S, D), np.float64)
    for c in range(8):
        out[c // 4] += res.results[c]["outp"].astype(np.float64)
    return out.astype(np.float32)
